# revision 34
# baseline (speedup 1.0000x reference)
"""MoE gate (sigmoid scores + grouped top-k routing) on 8 Trainium2 cores.

Reference computation (per token):
    scores = sigmoid(x @ W.T)                  # [T, 256]
    s = scores + bias                          # selection scores
    group_score[g] = sum(top2(s[g*32:(g+1)*32]))
    keep top-4 groups, mask the rest to -inf
    idx = top8(masked s)                       # [T, 8] int32, descending
    w = scores[idx]; w = w / w.sum() * 2.5     # [T, 8] f32

Sharding: tokens split 8 ways (2048/core); W/bias replicated. Host
pre-shuffles x and W into the transposed tiled layout the TensorE needs
(contraction dim on partitions) so the device does no transposes.

Default mode 'fp16c': logits = fp16(x) @ fp16(w) main pass (1 cyc/row)
plus a fused fp8e4 DoubleRow correction pass (2 rows/cycle) carrying
the two first-order quantization residuals, K-concatenated:
[e4m3(2^12*(x-fp16 x)) ; e4m3(x)] @ [e4m3(2^4*w) ; e4m3(2^16*(w-fp16 w))].
Both products land at 2^16x natural scale in a second PSUM; ScalarE
downscales, DVE adds to the main PSUM. ~15-bit effective logits at
2/3 the TensorE cost of bf16x3 (the top-k index flip count vs the fp32
reference stays at the HW accumulation-noise floor, ~18 of 131072
entries, metric ~8e-3 vs the 2e-2 gate).

DMA: 2KB-line column chunks (the DMA engines' throughput sweet spot;
14KB few-partition lines run at half rate), issued from BOTH hardware
DGE queues — main operands on the sync engine, correction operands on
the activation engine — to overlap the ~625ns-per-issue descriptor
cost that otherwise starves the 16 DMA engines.

Device per 128-token tile: 56 fp16 matmuls + 56 fp8-DR matmuls into
PSUM -> combine + sigmoid -> routing (grouped top-2 via segmented
reduce_max + match_replace, group top-4 via max8, expert top-8 via
max8/find_index8) on VectorE. Final score gather is 8 fused
compare-multiply-accumulate ops.
"""

import os

import numpy as np

import concourse.bass as bass
import concourse.mybir as mybir
import concourse.tile as tile
from concourse import bacc
from concourse.bass_utils import run_bass_kernel_spmd

T = 16384
DIM = 7168
E = 256
G = 8
EPG = E // G          # 32 experts per group
TOPKG = 4
TOPK = 8
SCALE = 2.5
NCORES = 8
TPC = T // NCORES     # 2048 tokens per core
P = 128
NT = TPC // P         # 16 token tiles per core
KT = DIM // P         # 56 contraction tiles
NEG = -1.0e30

# matmul precision: 'fp16c' (fp16 main + fused fp8e4 DoubleRow correction,
# ~15-bit effective at 2 cyc/row-equivalent), 'fp16'/'bf16' (1 cyc/row,
# single pass), 'fp32r', 'bf16x3' (3 cyc/row, near-fp32), 'fp32' (4 cyc/row)
MODE = os.environ.get("GATE_KERNEL_MODE", "fp16c")

# correction operand scales for fp16c (see kernel() for derivation)
XL_SCALE = 2.0 ** 12   # xl8 = e4m3(XL_SCALE * (x - fp16(x)))
WH_SCALE = 2.0 ** 4    # wh8 = e4m3(WH_SCALE * w)
WL_SCALE = 2.0 ** 16   # wl8 = e4m3(WL_SCALE * (w - fp16(w)))
CORR_SCALE = 1.0 / (XL_SCALE * WH_SCALE)  # psum_corr -> natural units

f32 = mybir.dt.float32
f32r = mybir.dt.float32r
f16 = mybir.dt.float16
bf16 = mybir.dt.bfloat16
f8e4 = mybir.dt.float8e4
i32 = mybir.dt.int32
u16 = mybir.dt.uint16
Alu = mybir.AluOpType
Act = mybir.ActivationFunctionType
AxX = mybir.AxisListType.X

last_run = {}


def _build(mode):
    nc = bacc.Bacc("TRN2", target_bir_lowering=False, debug=False,
                   num_devices=NCORES)

    if mode == "bf16x3":
        xhi = nc.dram_tensor("xhi", [NT, P, DIM], bf16, kind="ExternalInput").ap()
        xlo = nc.dram_tensor("xlo", [NT, P, DIM], bf16, kind="ExternalInput").ap()
        whi = nc.dram_tensor("whi", [P, KT * E], bf16, kind="ExternalInput").ap()
        wlo = nc.dram_tensor("wlo", [P, KT * E], bf16, kind="ExternalInput").ap()
    elif mode == "fp16c":
        xt = nc.dram_tensor("xt", [NT, P, DIM], f16, kind="ExternalInput").ap()
        xcor = nc.dram_tensor("xcor", [NT, P, 2 * DIM], f8e4,
                              kind="ExternalInput").ap()
        wt = nc.dram_tensor("wt", [P, KT * E], f16, kind="ExternalInput").ap()
        wcor = nc.dram_tensor("wcor", [P, 2 * KT * E], f8e4,
                              kind="ExternalInput").ap()
    else:
        xdt = {"fp32r": f32r, "fp32": f32, "fp16": f16, "bf16": bf16}[mode]
        xt = nc.dram_tensor("xt", [NT, P, DIM], xdt, kind="ExternalInput").ap()
        wt = nc.dram_tensor("wt", [P, KT * E], xdt, kind="ExternalInput").ap()
    biasb = nc.dram_tensor("biasb", [P, E], f32, kind="ExternalInput").ap()
    w_out = nc.dram_tensor("w_out", [TPC, TOPK], f32, kind="ExternalOutput").ap()
    idx_out = nc.dram_tensor("idx_out", [TPC, TOPK], i32, kind="ExternalOutput").ap()

    with tile.TileContext(nc) as tc:
        with (
            tc.tile_pool(name="const", bufs=1) as const,
            tc.tile_pool(name="xp", bufs=3) as xp,
            tc.tile_pool(name="ps", bufs=6 if mode != "fp16c" else 3,
                         space="PSUM") as psp,
            tc.tile_pool(name="rt", bufs=3) as rt,
        ):
            # PE warmup: dummy matmuls on zeroed scratch with no DMA deps —
            # trips the HAM clock gate to 2.4 GHz while the first tiles
            # stream in
            # memset on gpsimd (its preamble ends ~1.4us before DVE's) and
            # 128-wide warm matmuls so the ramp starts sooner and real
            # matmuls take over earlier
            warm_sb = const.tile([P, P], bf16, tag="warm")
            nc.gpsimd.memset(warm_sb[:], 0.0)
            with tc.tile_pool(name="warmps", bufs=1, space="PSUM") as wpsp:
                warm_ps = wpsp.tile([P, P], f32)
                NWARM = int(os.environ.get("GATE_KERNEL_NWARM", "16"))
                for i in range(NWARM):
                    nc.tensor.matmul(warm_ps[:], warm_sb[:], warm_sb[:],
                                     start=(i == 0), stop=(i == NWARM - 1))

            # weight + tile-0 x loads, chunked and interleaved in K order so
            # the k=0 operands land first and matmuls start ~4us in; each
            # DMA stays on one queue so per-matmul waits stay within ISA
            # limits
            WCH = 8   # K-tiles per weight DMA chunk
            XCH = 8 * P  # x free-dim elements per DMA chunk
            if mode == "bf16x3":
                whi_t = const.tile([P, KT * E], bf16)
                wlo_t = const.tile([P, KT * E], bf16)
                xh0 = xp.tile([P, DIM], bf16, tag="xh")
                xl0 = xp.tile([P, DIM], bf16, tag="xl")
                for c in range(0, KT, WCH):
                    wsl = slice(c * E, (c + WCH) * E)
                    xsl = slice(c * P, (c + WCH) * P)
                    nc.sync.dma_start(whi_t[:, wsl], whi[:, wsl])
                    nc.sync.dma_start(wlo_t[:, wsl], wlo[:, wsl])
                    nc.sync.dma_start(xh0[:, xsl], xhi[0][:, xsl])
                    nc.sync.dma_start(xl0[:, xsl], xlo[0][:, xsl])
            elif mode == "fp16c":
                # main-pass w + x0 on the sync DGE (k-chunk interleaved so
                # tile-0 matmuls start early); correction operands issued
                # in parallel from the activation DGE
                wt_t = const.tile([P, KT * E], f16)
                wc_t = const.tile([P, 2 * KT * E], f8e4)
                xf0 = xp.tile([P, DIM], f16, tag="x")
                xc0 = xp.tile([P, 2 * DIM], f8e4, tag="xc")
                for c in range(0, KT, WCH):
                    wsl = slice(c * E, (c + WCH) * E)
                    xsl = slice(c * P, (c + WCH) * P)
                    nc.sync.dma_start(wt_t[:, wsl], wt[:, wsl])
                    nc.sync.dma_start(xf0[:, xsl], xt[0][:, xsl])
                for c in range(0, 2 * KT, 2 * WCH):
                    wsl = slice(c * E, (c + 2 * WCH) * E)
                    xsl = slice(c * P, (c + 2 * WCH) * P)
                    nc.scalar.dma_start(wc_t[:, wsl], wcor[:, wsl])
                    nc.scalar.dma_start(xc0[:, xsl], xcor[0][:, xsl])
            else:
                wt_t = const.tile([P, KT * E], xdt)
                xf0 = xp.tile([P, DIM], xdt, tag="x")
                for c in range(0, KT, WCH):
                    wsl = slice(c * E, (c + WCH) * E)
                    xsl = slice(c * P, (c + WCH) * P)
                    nc.sync.dma_start(wt_t[:, wsl], wt[:, wsl])
                    nc.sync.dma_start(xf0[:, xsl], xt[0][:, xsl])
            bias_t = const.tile([P, E], f32)
            nc.sync.dma_start(bias_t[:], biasb)

            def routing(tt, sin):
                # ---- sigmoid (-> SBUF) ----
                orig = rt.tile([P, E], f32, tag="orig")
                nc.scalar.activation(orig[:], sin[:], Act.Sigmoid)

                # ---- selection scores s = orig + bias ----
                s = rt.tile([P, E], f32, tag="s")
                nc.vector.tensor_tensor(s[:], orig[:], bias_t[:], Alu.add)
                s3 = s[:].rearrange("p (g j) -> p g j", g=G)

                # ---- per-group top-2 sum ----
                m1 = rt.tile([P, G], f32, tag="m1")
                nc.vector.tensor_reduce(m1[:], s3, AxX, Alu.max)
                srep = rt.tile([P, E], f32, tag="srep")
                nc.vector.match_replace(srep[:], m1[:], s[:], NEG)
                m2 = rt.tile([P, G], f32, tag="m2")
                nc.vector.tensor_reduce(
                    m2[:], srep[:].rearrange("p (g j) -> p g j", g=G), AxX, Alu.max)
                gs = rt.tile([P, G], f32, tag="gs")
                nc.vector.tensor_tensor(gs[:], m1[:], m2[:], Alu.add)

                # ---- top-4 groups: threshold = 4th largest group score ----
                gtop = rt.tile([P, 8], f32, tag="gtop")
                nc.vector.max(gtop[:], gs[:])
                km = rt.tile([P, G], f32, tag="km")  # 0 kept, NEG dropped
                nc.vector.tensor_scalar(
                    km[:], gs[:], gtop[:, TOPKG - 1:TOPKG], NEG,
                    op0=Alu.is_lt, op1=Alu.mult)

                # ---- mask dropped groups: smask = s + km[group] ----
                # one broadcast add (km stride-0 over the 32 experts per
                # group); measured ~100ns vs 1.9us for 8 per-group ops
                smask = rt.tile([P, E], f32, tag="smask")
                kmb = km[:].unsqueeze(2).to_broadcast([P, G, EPG])
                nc.vector.tensor_tensor(
                    smask[:].rearrange("p (g j) -> p g j", g=G),
                    s3, kmb, Alu.add)

                # ---- expert top-8 values + indices ----
                v8 = rt.tile([P, TOPK], f32, tag="v8")
                nc.vector.max(v8[:], smask[:])
                i8u = rt.tile([P, TOPK], u16, tag="i8u")
                nc.vector.max_index(i8u[:], v8[:], smask[:])

                # ---- gather original scores at the 8 winners ----
                w8r = rt.tile([P, TOPK], f32, tag="w8r")
                for k in range(TOPK):
                    tmp = rt.tile([P, E], f32, tag="tmp")
                    nc.vector.scalar_tensor_tensor(
                        tmp[:], smask[:], v8[:, k:k + 1], orig[:],
                        op0=Alu.is_equal, op1=Alu.mult,
                        accum_out=w8r[:, k:k + 1])

                # ---- normalize * SCALE ----
                ssum = rt.tile([P, 1], f32, tag="ssum")
                nc.vector.tensor_reduce(ssum[:], w8r[:], AxX, Alu.add)
                rec = rt.tile([P, 1], f32, tag="rec")
                nc.vector.reciprocal(rec[:], ssum[:])
                w8 = rt.tile([P, TOPK], f32, tag="w8")
                nc.vector.tensor_scalar(
                    w8[:], w8r[:], rec[:, 0:1], SCALE,
                    op0=Alu.mult, op1=Alu.mult)
                i8 = rt.tile([P, TOPK], i32, tag="i8")
                nc.vector.tensor_copy(i8[:], i8u[:])

                nc.sync.dma_start(w_out[tt * P:(tt + 1) * P, :], w8[:])
                nc.sync.dma_start(idx_out[tt * P:(tt + 1) * P, :], i8[:])

            if mode == "fp16c":
                # software pipeline: correction matmuls (and routing) for
                # tile tt run after main matmuls of tile tt+2, so the
                # startup burst of correction-operand DMA (wc + xc) never
                # stalls the PE
                DEFER = 0
                xcs, pss = {}, {}
                for step in range(NT + DEFER):
                    if step < NT:
                        if step == 0:
                            xf, xc = xf0, xc0
                        else:
                            # 128-partition x 2KB-line chunks; main x on
                            # the sync DGE, correction x on the act DGE
                            xf = xp.tile([P, DIM], f16, tag="x")
                            xc = xp.tile([P, 2 * DIM], f8e4, tag="xc")
                            for c in range(0, DIM, XCH):
                                sl = slice(c, c + XCH)
                                nc.sync.dma_start(xf[:, sl], xt[step][:, sl])
                            for c in range(0, 2 * DIM, 2 * XCH):
                                sl = slice(c, c + 2 * XCH)
                                nc.scalar.dma_start(xc[:, sl],
                                                    xcor[step][:, sl])
                        ps = psp.tile([P, E], f32)
                        for k in range(KT):
                            nc.tensor.matmul(ps[:], xf[:, k * P:(k + 1) * P],
                                             wt_t[:, k * E:(k + 1) * E],
                                             start=(k == 0),
                                             stop=(k == KT - 1))
                        xcs[step], pss[step] = xc, ps
                    if step >= DEFER:
                        tt = step - DEFER
                        xc_t, ps_t = xcs.pop(tt), pss.pop(tt)
                        # fused fp8 correction: 2*KT k-subtiles (xl8||xh8
                        # vs wh8||wl8), two per DoubleRow matmul
                        psc = psp.tile([P, E], f32, tag="corr")
                        for i in range(KT):
                            lhs = xc_t[:, 2 * i * P:(2 * i + 2) * P].rearrange(
                                "p (two j) -> p two j", two=2)
                            rhs = wc_t[:, 2 * i * E:(2 * i + 2) * E].rearrange(
                                "p (two j) -> p two j", two=2)
                            nc.tensor.matmul(
                                psc[:], lhs, rhs,
                                start=(i == 0), stop=(i == KT - 1),
                                perf_mode=mybir.MatmulPerfMode.DoubleRow)
                        # logits = main + 2^-16 * corr. DVE can read only
                        # one PSUM input, so ScalarE downscales corr first
                        corr_sb = rt.tile([P, E], f32, tag="corr_sb")
                        nc.scalar.activation(corr_sb[:], psc[:], Act.Copy,
                                             scale=CORR_SCALE)
                        spre = rt.tile([P, E], f32, tag="spre")
                        nc.vector.tensor_tensor(spre[:], corr_sb[:], ps_t[:],
                                                Alu.add)
                        routing(tt, spre)
            else:
                for tt in range(NT):
                    # ---- load x tile (partition = contraction), chunked ----
                    if mode == "bf16x3":
                        if tt == 0:
                            xh, xl = xh0, xl0
                        else:
                            xh = xp.tile([P, DIM], bf16, tag="xh")
                            xl = xp.tile([P, DIM], bf16, tag="xl")
                            for c in range(0, DIM, XCH):
                                sl = slice(c, c + XCH)
                                nc.sync.dma_start(xh[:, sl], xhi[tt][:, sl])
                                nc.sync.dma_start(xl[:, sl], xlo[tt][:, sl])
                    else:
                        if tt == 0:
                            xf = xf0
                        else:
                            xf = xp.tile([P, DIM], xdt, tag="x")
                            for c in range(0, DIM, XCH):
                                sl = slice(c, c + XCH)
                                nc.sync.dma_start(xf[:, sl], xt[tt][:, sl])

                    # ---- logits: accumulate over 56 K-tiles into PSUM ----
                    ps = psp.tile([P, E], f32)
                    if mode == "bf16x3":
                        nmm = 3 * KT
                        i = 0
                        for k in range(KT):
                            for xs, ws in ((xh, whi_t), (xh, wlo_t),
                                           (xl, whi_t)):
                                nc.tensor.matmul(
                                    ps[:],
                                    xs[:, k * P:(k + 1) * P],
                                    ws[:, k * E:(k + 1) * E],
                                    start=(i == 0), stop=(i == nmm - 1),
                                )
                                i += 1
                    else:
                        for k in range(KT):
                            nc.tensor.matmul(ps[:], xf[:, k * P:(k + 1) * P],
                                             wt_t[:, k * E:(k + 1) * E],
                                             start=(k == 0),
                                             stop=(k == KT - 1))
                    routing(tt, ps)

    nc.compile()
    return nc


def _shuffle_x(xc):
    """[TPC, DIM] -> [NT, P, DIM] with out[tt, p, k*128+j] = xc[tt*128+j, k*128+p]."""
    return np.ascontiguousarray(
        xc.reshape(NT, P, KT, P).transpose(0, 3, 2, 1).reshape(NT, P, DIM))


def _shuffle_w(w):
    """[E, DIM] -> [P, KT*E] with out[p, k*E+e] = w[e, k*128+p]."""
    return np.ascontiguousarray(
        w.T.reshape(KT, P, E).transpose(1, 0, 2).reshape(P, KT * E))


_nc_cache = {}


def kernel(x, weight, bias):
    import ml_dtypes

    x = np.asarray(x, dtype=np.float32)
    weight = np.asarray(weight, dtype=np.float32)
    bias = np.asarray(bias, dtype=np.float32)

    mode = MODE
    if mode not in _nc_cache:
        _nc_cache[mode] = _build(mode)
    nc = _nc_cache[mode]

    biasb = np.ascontiguousarray(np.broadcast_to(bias, (P, E)))
    in_maps = []
    if mode == "bf16x3":
        w_hi = weight.astype(ml_dtypes.bfloat16)
        w_lo = (weight - w_hi.astype(np.float32)).astype(ml_dtypes.bfloat16)
        whi = _shuffle_w(w_hi.astype(np.float32)).astype(ml_dtypes.bfloat16)
        wlo = _shuffle_w(w_lo.astype(np.float32)).astype(ml_dtypes.bfloat16)
        for c in range(NCORES):
            xc = x[c * TPC:(c + 1) * TPC]
            x_hi = xc.astype(ml_dtypes.bfloat16)
            x_lo = (xc - x_hi.astype(np.float32)).astype(ml_dtypes.bfloat16)
            in_maps.append({
                "xhi": _shuffle_x(x_hi.astype(np.float32)).astype(ml_dtypes.bfloat16),
                "xlo": _shuffle_x(x_lo.astype(np.float32)).astype(ml_dtypes.bfloat16),
                "whi": whi, "wlo": wlo, "biasb": biasb,
            })
    elif mode == "fp16c":
        f8 = ml_dtypes.float8_e4m3
        wh16 = weight.astype(np.float16).astype(np.float32)
        wl = weight - wh16
        # correction pieces (values carry their scale; PSUM = 2^16 * corr)
        wh8 = (weight * WH_SCALE).astype(f8).astype(np.float32)
        wl8 = (wl * WL_SCALE).astype(f8).astype(np.float32)
        wt_h = _shuffle_w(wh16).astype(np.float16)
        wc_h = np.concatenate(
            [_shuffle_w(wh8), _shuffle_w(wl8)], axis=1).astype(f8)
        for c in range(NCORES):
            xc = x[c * TPC:(c + 1) * TPC]
            xh16 = xc.astype(np.float16).astype(np.float32)
            xl8 = ((xc - xh16) * XL_SCALE).astype(f8).astype(np.float32)
            xh8 = xc.astype(f8).astype(np.float32)
            xcor_h = np.concatenate(
                [_shuffle_x(xl8), _shuffle_x(xh8)], axis=2).astype(f8)
            in_maps.append({"xt": _shuffle_x(xc).astype(np.float16),
                            "xcor": xcor_h, "wt": wt_h, "wcor": wc_h,
                            "biasb": biasb})
    else:
        np_dt = {"fp16": np.float16, "bf16": ml_dtypes.bfloat16}.get(
            mode, np.float32)
        wt = _shuffle_w(weight).astype(np_dt)
        for c in range(NCORES):
            xc = x[c * TPC:(c + 1) * TPC]
            in_maps.append({"xt": _shuffle_x(xc).astype(np_dt), "wt": wt,
                            "biasb": biasb})

    trace = bool(int(os.environ.get("GATE_KERNEL_TRACE", "0")))
    res = run_bass_kernel_spmd(nc, in_maps, core_ids=list(range(NCORES)),
                               trace=trace)
    last_run["exec_time_ns"] = res.exec_time_ns
    last_run["mean_exec_time_ns"] = res.mean_exec_time_ns
    last_run["trace"] = res.instructions_and_trace

    w = np.concatenate([res.results[c]["w_out"] for c in range(NCORES)], axis=0)
    idx = np.concatenate([res.results[c]["idx_out"] for c in range(NCORES)], axis=0)
    return w.astype(np.float32), idx.astype(np.int32)

